# revision 39
# baseline (speedup 1.0000x reference)
"""Trainium2 Bass kernel for nn_Attention_41704132444382 (v7, ~238us).

Masked-linear QKV + 16-head attention + masked-linear output projection,
tensor-parallel over heads across 8 NeuronCores (2 heads/core).

Structure (vs the 265us v3 baseline):
  - Softmax normalization fused on-device into the PV-psum evacuation:
    per-head V tiles carry 64 ones-columns, so the PV matmul replicates
    the softmax denominator across psum rows 64..127 at zero PE cost
    (PE time scales with output free size only, not partitions).
    1/den via int-magic seed (0x7EF311C3 - bits) + one Newton step in
    3 standard DVE ops (tensor_scalar + 2 scalar_tensor_tensor), then
    one tensor_tensor mult per head normalizes during evacuation
    (max den err ~0.26%). nc.vector.reciprocal is ~6 cycles/elem (3.3us
    per call!) and custom DVE ops (reciprocal_approx_fast) miscompile
    under the axon jit path - avoid both.
  - With `at` pre-normalized, the out-projection contracts all 128
    head-dims of BOTH heads in one K=128 matmul per (token-group,
    oh-half): half of v3's out-proj PE work, heads summed in psum for
    free, po output halves to [T, DIM]. Host only sums 8 cores + bias.
  - dtypes: x/wqkv/qT/kTt/at/wo/po fp16 (halves x DMA to 8MB and makes
    score matmuls full-rate: PE moving-data streams 2B/cycle/partition,
    so f32r runs half-speed), e/v bf16 (exp values span 1e+-26; fp16
    overflows). Scores stay f32 psum.
  - exp split Scalar/DVE: BASS_ATTN_SCHRAUD_JT j-tiles per block
    (default 6) use a DVE Schraudolph exp (int16(A*s+B) viewed as bf16,
    ~3% elementwise, softmax num/den-correlated so final err ~1%);
    the rest are exact ScalarE ACTIVATEs. The per-jt cadence is gated
    by exp latency through the 2-deep score psum ring, so both engines
    must serve exps concurrently.
  - PSUM (8 banks exactly): scores 2x[128,1024] (4) + pv 3x[128,512]
    (3, triple-buffered so the next block's PV never waits the normalize
    chain) + po 1x[128,512] (1, sequential oh-halves). The po halves
    free the bank that makes pvps triple-buffering fit.
  - po for block b runs during block b+1 at jts 3,5,7,9 as sequential
    1-bank oh-halves; evacs alternate ScalarE/DVE (BASS_ATTN_PO_SCALAR).
    The trailing (last-block) po instead uses wide 2-bank tiles from the
    then-idle scores pool to avoid ring serialization.
  - Phase 1: ALL 32 x tiles resident (xq bufs=32, 64KB/partition) so
    every DMA can issue immediately - with a smaller pool the issues
    themselves stall on pool-slot WAR deps and the chains run at low
    p-state. x striped across the gpsimd+scalar queues per-tile;
    weights first (small, every chain needs them).
  - V^T -> V transposes via DMA crossbar on the sync queue, emitted
    mid-phase-1 as soon as each batch's V columns are complete; the
    sync queue must carry only transposes until they finish.
"""

import math
import os
import sys

import numpy as np

sys.path.insert(0, "/opt/trn_rl_repo")

import concourse.bass as bass
import concourse.mybir as mybir
from concourse import bacc
from concourse.tile import TileContext

DIM = 1024
HEADS = 16
B = 2
N = 2048
T = B * N  # 4096 flattened tokens
NCORES = 8
DV = 128  # head-dims per core (2 heads x 64)
SCALE = DIM ** (-0.5)  # 1/32

F32 = mybir.dt.float32
F32R = mybir.dt.float32r
F16 = mybir.dt.float16
BF16 = mybir.dt.bfloat16
I16 = mybir.dt.int16
I32 = mybir.dt.int32
RECIP_K = 0x7EF311C3

# number of j-tiles (of 16) per block whose exp runs on DVE (Schraudolph)
N_SCHRAUD = int(os.environ.get("BASS_ATTN_SCHRAUD_JT", "6"))
# of the 4 po evacuations per block, how many run on ScalarE (rest DVE)
PO_SCALAR = int(os.environ.get("BASS_ATTN_PO_SCALAR", "2"))
# bf16-bits variant: int16(A16*s + B16) viewed as bf16
SCHRAUD_A = (2.0 ** 7) / math.log(2.0) * SCALE
SCHRAUD_B = 127.0 * 128.0 - float(os.environ.get("BASS_ATTN_SCHRAUD_SIGMA", "7.42"))


def build_nc():
    nc = bacc.Bacc("TRN2", target_bir_lowering=True)
    xT_d = nc.declare_dram_parameter("xT", [DIM, T], F16, isOutput=False)
    wqkvT_d = nc.declare_dram_parameter("wqkvT", [DIM, 384], F16, isOutput=False)
    woT_d = nc.declare_dram_parameter("woT", [DV, DIM], F16, isOutput=False)
    po_d = nc.declare_dram_parameter("po", [T, DIM], F16, isOutput=True)

    mult = mybir.AluOpType.mult
    add = mybir.AluOpType.add
    Exp = mybir.ActivationFunctionType.Exp
    Copy = mybir.ActivationFunctionType.Copy
    Recip = mybir.ActivationFunctionType.Reciprocal

    # spread the DVE-exp tiles across the block (odd tiles first, then even)
    order = [1, 3, 5, 7, 9, 11, 13, 15, 2, 4, 6, 8, 10, 12, 14, 0]
    schraud_jt = set(order[: min(N_SCHRAUD, 16)])

    with TileContext(nc) as tc:
        with (
            tc.tile_pool(name="persist", bufs=1) as pp,
            tc.tile_pool(name="vstg", bufs=2) as vsp,
        ):
            wqkv_g = pp.tile([128, 8 * 384], F16)  # [k-part, (kt, o)]
            wo_g = pp.tile([128, 1024], F16)
            qT = pp.tile([128, 4096], F16)
            kTt = pp.tile([128, 4096], F16)
            vT = pp.tile([128, 4096], BF16)
            # per j-tile blocks of 128: cols 0..63 = V, cols 64..127 = ones
            v1 = pp.tile([128, 32 * 128], BF16)  # head 0
            v2 = pp.tile([128, 32 * 128], BF16)  # head 1

            def emit_vtransp(b):
                # V^T [dv, t] -> V [t, dv] via DMA crossbar; contiguous
                # staging then strided copy into the (dv|ones) layout
                for h, vv in enumerate((v1, v2)):
                    vstg = vsp.tile([128, 1024], BF16, tag="vstg")
                    nc.sync.dma_start_transpose(
                        vstg[:].rearrange("p (j c) -> p j c", c=64),
                        vT[h * 64 : (h + 1) * 64, b * 2048 : (b + 1) * 2048],
                    )
                    nc.vector.tensor_copy(
                        vv[:].rearrange("p (j c) -> p j c", c=128)[
                            :, b * 16 : (b + 1) * 16, 0:64
                        ],
                        vstg[:].rearrange("p (j c) -> p j c", c=64),
                    )

            # ---------- Phase 1: QKV projection ----------
            with (
                tc.tile_pool(name="xq", bufs=32) as xp,
                tc.tile_pool(name="qk_ps", bufs=4, space="PSUM") as qkps,
            ):
                xq_tiles = {}

                def load_quarter(q, eng=None):
                    xq_tiles[q] = [
                        xp.tile([128, 1024], F16, tag="xq", name=f"xq{q}_{i}")
                        for i in range(8)
                    ]
                    for kt in range(8):
                        e = nc.gpsimd if kt % 2 == 0 else nc.scalar
                        e.dma_start(
                            xq_tiles[q][kt][:],
                            xT_d[kt * 128 : (kt + 1) * 128, q * 1024 : (q + 1) * 1024],
                        )

                DESTS = (qT, kTt, vT)

                def emit_chain(q, ot, th):
                    ps = qkps.tile([128, 512], F32, tag="qkps", name=f"qk{q}_{ot}_{th}")
                    for kt in range(8):
                        nc.tensor.matmul(
                            ps[:],
                            wqkv_g[:, kt * 384 + ot * 128 : kt * 384 + (ot + 1) * 128],
                            xq_tiles[q][kt][:, th * 512 : (th + 1) * 512],
                            start=(kt == 0),
                            stop=(kt == 7),
                        )
                    col = q * 1024 + th * 512
                    nc.vector.tensor_copy(DESTS[ot][:, col : col + 512], ps[:])

                # weights first (small, needed by every chain), then x
                nc.scalar.dma_start(
                    wqkv_g[:].rearrange("p (kt o) -> p kt o", kt=8),
                    wqkvT_d[:].rearrange("(kt p) o -> p kt o", p=128),
                )
                load_quarter(0)
                load_quarter(1)
                nc.scalar.dma_start(wo_g[:], woT_d[:])
                load_quarter(2)
                load_quarter(3)
                for vv in (v1, v2):
                    nc.vector.memset(
                        vv[:].rearrange("p (j c) -> p j c", c=128)[:, :, 64:128],
                        1.0,
                    )
                for q in range(4):
                    for ot in range(3):
                        for th in range(2):
                            emit_chain(q, ot, th)
                    if q == 1:
                        emit_vtransp(0)  # vT[:, 0:2048] complete
                    elif q == 3:
                        emit_vtransp(1)

            # ---------- Phase 2: attention ----------
            with (
                tc.tile_pool(name="es", bufs=6) as ep,
                tc.tile_pool(name="at", bufs=2) as atp,
                tc.tile_pool(name="rc", bufs=6) as rcp,
                tc.tile_pool(name="ob", bufs=4) as obp,
                tc.tile_pool(name="s_ps", bufs=2, space="PSUM") as sps,
                tc.tile_pool(name="pv_ps", bufs=3, space="PSUM") as pvps,
                tc.tile_pool(name="po_ps", bufs=1, space="PSUM") as pops,
            ):
                def emit_po(prev, tg):
                    # one K=128 matmul per oh-half through a 1-bank psum tile
                    # (sequential halves free a bank for pvps triple-buffering)
                    pb, at = prev
                    row = pb * 512 + tg * 128
                    for oh in range(2):
                        po = pops.tile(
                            [128, 512], F32, tag="po", name=f"po{pb}_{tg}_{oh}"
                        )
                        nc.tensor.matmul(
                            po[:],
                            at[:, tg * 128 : (tg + 1) * 128],
                            wo_g[:, oh * 512 : (oh + 1) * 512],
                            start=True,
                            stop=True,
                        )
                        ob = obp.tile(
                            [128, 512], F16, tag="ob", name=f"ob{pb}_{tg}_{oh}"
                        )
                        if (tg + oh) % 2 == 0:
                            nc.scalar.activation(ob[:], po[:], Copy)
                        else:
                            nc.vector.tensor_copy(ob[:], po[:])
                        nc.sync.dma_start(
                            po_d[row : row + 128, oh * 512 : (oh + 1) * 512], ob[:]
                        )

                # ---- attention blocks ----
                prev = None
                for bb in range(8):
                    b, ic = bb // 4, bb % 4
                    i0 = b * 2048 + ic * 512
                    pvs = [
                        pvps.tile([128, 512], F32, tag="pv", name=f"pv{bb}_{h}")
                        for h in range(2)
                    ]
                    for jt in range(16):
                        j0 = b * 2048 + jt * 128
                        jv = (b * 16 + jt) * 128
                        sg = sps.tile([128, 1024], F32, tag="s", name=f"s{bb}_{jt}")
                        for h in range(2):
                            nc.tensor.matmul(
                                sg[:, h * 512 : (h + 1) * 512],
                                kTt[h * 64 : (h + 1) * 64, j0 : j0 + 128],
                                qT[h * 64 : (h + 1) * 64, i0 : i0 + 512],
                                start=True,
                                stop=True,
                                tile_position=(h * 64, 0),
                            )
                        et = ep.tile([128, 1024], BF16, tag="e", name=f"e{bb}_{jt}")
                        if jt in schraud_jt:
                            nc.vector.tensor_scalar(
                                et[:].bitcast(I16),
                                sg[:],
                                SCHRAUD_A,
                                SCHRAUD_B,
                                mult,
                                add,
                            )
                        else:
                            nc.scalar.activation(et[:], sg[:], Exp, scale=SCALE)
                        for h, vv in enumerate((v1, v2)):
                            nc.tensor.matmul(
                                pvs[h][:],
                                vv[:, jv : jv + 128],
                                et[:, h * 512 : (h + 1) * 512],
                                start=(jt == 0),
                                stop=(jt == 15),
                            )
                        if prev is not None and jt in (3, 5, 7, 9):
                            emit_po(prev, (jt - 3) // 2)
                    # --- evacuate block: normalized attn^T via den rows ---
                    at = atp.tile([128, 512], F16, tag="at", name=f"at{bb}")
                    for h in range(2):
                        # rec = 1/den via int-magic seed + one Newton step
                        y0 = rcp.tile([64, 512], F32, tag="rc", name=f"y0{bb}_{h}")
                        nc.vector.tensor_scalar(
                            y0[:].bitcast(I32),
                            pvs[h][64:128, :].bitcast(I32),
                            -1,
                            RECIP_K,
                            mult,
                            add,
                        )
                        u = rcp.tile([64, 512], F32, tag="rc", name=f"u{bb}_{h}")
                        nc.vector.scalar_tensor_tensor(
                            u[:], pvs[h][64:128, :], -1.0, y0[:], mult, mult
                        )
                        rec = rcp.tile([64, 512], F32, tag="rc", name=f"rc{bb}_{h}")
                        nc.vector.scalar_tensor_tensor(
                            rec[:], u[:], 2.0, y0[:], add, mult
                        )
                        nc.vector.tensor_tensor(
                            at[h * 64 : (h + 1) * 64, :],
                            pvs[h][0:64, :],
                            rec[:],
                            mult,
                        )
                    prev = (bb, at)

                # trailing po: block 7's scores pool is idle, use wide
                # 2-bank tiles there to avoid the 1-bank ring serialization
                pb, at = prev
                for tg in range(4):
                    row = pb * 512 + tg * 128
                    po = sps.tile([128, 1024], F32, tag="s", name=f"pot{tg}")
                    for oh in range(2):
                        nc.tensor.matmul(
                            po[:, oh * 512 : (oh + 1) * 512],
                            at[:, tg * 128 : (tg + 1) * 128],
                            wo_g[:, oh * 512 : (oh + 1) * 512],
                            start=True,
                            stop=True,
                        )
                    ob = obp.tile([128, 1024], F16, tag="ob", name=f"obt{tg}")
                    if tg % 2 == 0:
                        nc.scalar.activation(ob[:], po[:], Copy)
                    else:
                        nc.vector.tensor_copy(ob[:], po[:])
                    nc.sync.dma_start(po_d[row : row + 128, :], ob[:])

    nc.compile()
    return nc


_NC = None


def _get_nc():
    global _NC
    if _NC is None:
        _NC = build_nc()
    return _NC


def _gate(mask):
    """Exact jax fp32 gate: sigmoid(m) > 0.5 (fp32 logistic rounding)."""
    mask = np.asarray(mask, dtype=np.float32)
    return (np.float32(1.0) / (np.float32(1.0) + np.exp(-mask))) > np.float32(0.5)


def make_in_maps(x, qkv_weight, qkv_weight_mask, out_weight, out_weight_mask):
    x = np.asarray(x, dtype=np.float32)
    wqkv = np.where(_gate(qkv_weight_mask), np.asarray(qkv_weight, np.float32), 0.0)
    wo = np.where(_gate(out_weight_mask), np.asarray(out_weight, np.float32), 0.0)

    xT = np.ascontiguousarray(x.reshape(T, DIM).T.astype(np.float16))
    in_maps = []
    for c in range(NCORES):
        r0 = c * DV
        sl = slice(r0, r0 + DV)
        w_shard = np.concatenate(
            [wqkv[sl], wqkv[DIM + r0 : DIM + r0 + DV], wqkv[2 * DIM + r0 : 2 * DIM + r0 + DV]],
            axis=0,
        )  # [384, 1024] rows = (q h0,h1 | k h0,h1 | v h0,h1)
        in_maps.append(
            {
                "xT": xT,
                "wqkvT": np.ascontiguousarray(w_shard.T.astype(np.float16)),
                "woT": np.ascontiguousarray(wo[:, sl].T.astype(np.float16)),
            }
        )
    return in_maps


LAST_RESULTS = None  # BassKernelResults of the most recent run (for profiling)


def kernel(
    x,
    qkv_weight,
    qkv_weight_mask,
    out_weight,
    out_weight_mask,
    out_bias,
    out_bias_mask,
    _trace=False,
    _tmpdir=None,
):
    global LAST_RESULTS
    from concourse.bass_utils import run_bass_kernel_spmd

    nc = _get_nc()
    in_maps = make_in_maps(x, qkv_weight, qkv_weight_mask, out_weight, out_weight_mask)
    res = run_bass_kernel_spmd(
        nc, in_maps, list(range(NCORES)), trace=_trace, tmpdir=_tmpdir
    )
    LAST_RESULTS = res
    out = np.zeros((T, DIM), dtype=np.float32)
    for r in res.results:
        out += np.asarray(r["po"]).astype(np.float32)
    out_bias = np.asarray(out_bias, dtype=np.float32)
    out += np.where(_gate(out_bias_mask), out_bias, 0.0)[None, :]
    return out.reshape(B, N, DIM)


# revision 40
# speedup vs baseline: 1.2965x; 1.2965x over previous
"""Trainium2 Bass kernel for nn_Attention_41704132444382 (v7, ~238us).

Masked-linear QKV + 16-head attention + masked-linear output projection,
tensor-parallel over heads across 8 NeuronCores (2 heads/core).

Structure (vs the 265us v3 baseline):
  - Softmax normalization fused on-device into the PV-psum evacuation:
    per-head V tiles carry 64 ones-columns, so the PV matmul replicates
    the softmax denominator across psum rows 64..127 at zero PE cost
    (PE time scales with output free size only, not partitions).
    1/den via int-magic seed (0x7EF311C3 - bits) + one Newton step in
    3 standard DVE ops (tensor_scalar + 2 scalar_tensor_tensor), then
    one tensor_tensor mult per head normalizes during evacuation
    (max den err ~0.26%). nc.vector.reciprocal is ~6 cycles/elem (3.3us
    per call!) and custom DVE ops (reciprocal_approx_fast) miscompile
    under the axon jit path - avoid both.
  - With `at` pre-normalized, the out-projection contracts all 128
    head-dims of BOTH heads in one K=128 matmul per (token-group,
    oh-half): half of v3's out-proj PE work, heads summed in psum for
    free, po output halves to [T, DIM]. Host only sums 8 cores + bias.
  - dtypes: x/wqkv/qT/kTt/at/wo/po fp16 (halves x DMA to 8MB and makes
    score matmuls full-rate: PE moving-data streams 2B/cycle/partition,
    so f32r runs half-speed), e/v bf16 (exp values span 1e+-26; fp16
    overflows). Scores stay f32 psum.
  - exp split Scalar/DVE: BASS_ATTN_SCHRAUD_JT j-tiles per block
    (default 6) use a DVE Schraudolph exp (int16(A*s+B) viewed as bf16,
    ~3% elementwise, softmax num/den-correlated so final err ~1%);
    the rest are exact ScalarE ACTIVATEs. The per-jt cadence is gated
    by exp latency through the 2-deep score psum ring, so both engines
    must serve exps concurrently.
  - PSUM (8 banks exactly): scores 2x[128,1024] (4) + pv 3x[128,512]
    (3, triple-buffered so the next block's PV never waits the normalize
    chain) + po 1x[128,512] (1, sequential oh-halves). The po halves
    free the bank that makes pvps triple-buffering fit.
  - po for block b runs during block b+1 at jts 3,5,7,9 as sequential
    1-bank oh-halves; evacs alternate ScalarE/DVE (BASS_ATTN_PO_SCALAR).
    The trailing (last-block) po instead uses wide 2-bank tiles from the
    then-idle scores pool to avoid ring serialization.
  - Phase 1: ALL 32 x tiles resident (xq bufs=32, 64KB/partition) so
    every DMA can issue immediately - with a smaller pool the issues
    themselves stall on pool-slot WAR deps and the chains run at low
    p-state. x striped across the gpsimd+scalar queues per-tile;
    weights first (small, every chain needs them).
  - V^T -> V transposes via DMA crossbar on the sync queue, emitted
    mid-phase-1 as soon as each batch's V columns are complete; the
    sync queue must carry only transposes until they finish.
"""

import math
import os
import sys

import numpy as np

sys.path.insert(0, "/opt/trn_rl_repo")

import concourse.bass as bass
import concourse.mybir as mybir
from concourse import bacc
from concourse.tile import TileContext

DIM = 1024
HEADS = 16
B = 2
N = 2048
T = B * N  # 4096 flattened tokens
NCORES = 8
DV = 128  # head-dims per core (2 heads x 64)
SCALE = DIM ** (-0.5)  # 1/32

F32 = mybir.dt.float32
F32R = mybir.dt.float32r
F16 = mybir.dt.float16
BF16 = mybir.dt.bfloat16
I16 = mybir.dt.int16
I32 = mybir.dt.int32
RECIP_K = 0x7EF311C3

# number of j-tiles (of 16) per block whose exp runs on DVE (Schraudolph)
N_SCHRAUD = int(os.environ.get("BASS_ATTN_SCHRAUD_JT", "6"))
# of the 4 po evacuations per block, how many run on ScalarE (rest DVE)
PO_SCALAR = int(os.environ.get("BASS_ATTN_PO_SCALAR", "2"))
# bf16-bits variant: int16(A16*s + B16) viewed as bf16
SCHRAUD_A = (2.0 ** 7) / math.log(2.0) * SCALE
SCHRAUD_B = 127.0 * 128.0 - float(os.environ.get("BASS_ATTN_SCHRAUD_SIGMA", "7.42"))


def build_nc():
    nc = bacc.Bacc("TRN2", target_bir_lowering=True)
    xT_d = nc.declare_dram_parameter("xT", [DIM, T], F16, isOutput=False)
    wqkvT_d = nc.declare_dram_parameter("wqkvT", [DIM, 384], F16, isOutput=False)
    woT_d = nc.declare_dram_parameter("woT", [DV, DIM], F16, isOutput=False)
    po_d = nc.declare_dram_parameter("po", [T, DIM], F16, isOutput=True)

    mult = mybir.AluOpType.mult
    add = mybir.AluOpType.add
    Exp = mybir.ActivationFunctionType.Exp
    Copy = mybir.ActivationFunctionType.Copy
    Recip = mybir.ActivationFunctionType.Reciprocal

    # spread the DVE-exp tiles across the block (odd tiles first, then even)
    order = [15, 13, 11, 9, 7, 5, 3, 1, 14, 12, 10, 8, 6, 4, 2, 0]
    schraud_jt = set(order[: min(N_SCHRAUD, 16)])

    with TileContext(nc) as tc:
        with (
            tc.tile_pool(name="persist", bufs=1) as pp,
            tc.tile_pool(name="vstg", bufs=2) as vsp,
        ):
            wqkv_g = pp.tile([128, 8 * 384], F16)  # [k-part, (kt, o)]
            wo_g = pp.tile([128, 1024], F16)
            qT = pp.tile([128, 4096], F16)
            kTt = pp.tile([128, 4096], F16)
            vT = pp.tile([128, 4096], BF16)
            # per j-tile blocks of 128: cols 0..63 = V, cols 64..127 = ones
            v1 = pp.tile([128, 32 * 128], BF16)  # head 0
            v2 = pp.tile([128, 32 * 128], BF16)  # head 1

            def emit_vtransp(b):
                # V^T [dv, t] -> V [t, dv] via DMA crossbar; contiguous
                # staging then strided copy into the (dv|ones) layout
                for h, vv in enumerate((v1, v2)):
                    vstg = vsp.tile([128, 1024], BF16, tag="vstg")
                    nc.sync.dma_start_transpose(
                        vstg[:].rearrange("p (j c) -> p j c", c=64),
                        vT[h * 64 : (h + 1) * 64, b * 2048 : (b + 1) * 2048],
                    )
                    nc.vector.tensor_copy(
                        vv[:].rearrange("p (j c) -> p j c", c=128)[
                            :, b * 16 : (b + 1) * 16, 0:64
                        ],
                        vstg[:].rearrange("p (j c) -> p j c", c=64),
                    )

            # ---------- Phase 1: QKV projection ----------
            with (
                tc.tile_pool(name="xq", bufs=32) as xp,
                tc.tile_pool(name="qk_ps", bufs=4, space="PSUM") as qkps,
            ):
                xq_tiles = {}

                def load_quarter(q, eng=None):
                    xq_tiles[q] = [
                        xp.tile([128, 1024], F16, tag="xq", name=f"xq{q}_{i}")
                        for i in range(8)
                    ]
                    for kt in range(8):
                        e = nc.gpsimd if kt % 2 == 0 else nc.scalar
                        e.dma_start(
                            xq_tiles[q][kt][:],
                            xT_d[kt * 128 : (kt + 1) * 128, q * 1024 : (q + 1) * 1024],
                        )

                DESTS = (qT, kTt, vT)

                def emit_chain(q, ot, th):
                    ps = qkps.tile([128, 512], F32, tag="qkps", name=f"qk{q}_{ot}_{th}")
                    for kt in range(8):
                        nc.tensor.matmul(
                            ps[:],
                            wqkv_g[:, kt * 384 + ot * 128 : kt * 384 + (ot + 1) * 128],
                            xq_tiles[q][kt][:, th * 512 : (th + 1) * 512],
                            start=(kt == 0),
                            stop=(kt == 7),
                        )
                    col = q * 1024 + th * 512
                    nc.vector.tensor_copy(DESTS[ot][:, col : col + 512], ps[:])

                # weights first (small, needed by every chain), then x
                nc.scalar.dma_start(
                    wqkv_g[:].rearrange("p (kt o) -> p kt o", kt=8),
                    wqkvT_d[:].rearrange("(kt p) o -> p kt o", p=128),
                )
                load_quarter(0)
                load_quarter(1)
                nc.scalar.dma_start(wo_g[:], woT_d[:])
                load_quarter(2)
                load_quarter(3)
                for vv in (v1, v2):
                    nc.vector.memset(
                        vv[:].rearrange("p (j c) -> p j c", c=128)[:, :, 64:128],
                        1.0,
                    )
                for q in range(4):
                    for ot in range(3):
                        for th in range(2):
                            emit_chain(q, ot, th)
                    if q == 1:
                        emit_vtransp(0)  # vT[:, 0:2048] complete
                    elif q == 3:
                        emit_vtransp(1)

            # ---------- Phase 2: attention ----------
            with (
                tc.tile_pool(name="es", bufs=6) as ep,
                tc.tile_pool(name="at", bufs=2) as atp,
                tc.tile_pool(name="rc", bufs=6) as rcp,
                tc.tile_pool(name="ob", bufs=4) as obp,
                tc.tile_pool(name="s_ps", bufs=2, space="PSUM") as sps,
                tc.tile_pool(name="pv_ps", bufs=3, space="PSUM") as pvps,
                tc.tile_pool(name="po_ps", bufs=1, space="PSUM") as pops,
            ):
                def emit_po(prev, tg):
                    # one K=128 matmul per oh-half through a 1-bank psum tile
                    # (sequential halves free a bank for pvps triple-buffering)
                    pb, at = prev
                    row = pb * 512 + tg * 128
                    for oh in range(2):
                        po = pops.tile(
                            [128, 512], F32, tag="po", name=f"po{pb}_{tg}_{oh}"
                        )
                        nc.tensor.matmul(
                            po[:],
                            at[:, tg * 128 : (tg + 1) * 128],
                            wo_g[:, oh * 512 : (oh + 1) * 512],
                            start=True,
                            stop=True,
                        )
                        ob = obp.tile(
                            [128, 512], F16, tag="ob", name=f"ob{pb}_{tg}_{oh}"
                        )
                        if (tg + oh) % 2 == 0:
                            nc.scalar.activation(ob[:], po[:], Copy)
                        else:
                            nc.vector.tensor_copy(ob[:], po[:])
                        nc.sync.dma_start(
                            po_d[row : row + 128, oh * 512 : (oh + 1) * 512], ob[:]
                        )

                # ---- attention blocks ----
                prev = None
                for bb in range(8):
                    b, ic = bb // 4, bb % 4
                    i0 = b * 2048 + ic * 512
                    pvs = [
                        pvps.tile([128, 512], F32, tag="pv", name=f"pv{bb}_{h}")
                        for h in range(2)
                    ]
                    for jt in range(16):
                        j0 = b * 2048 + jt * 128
                        jv = (b * 16 + jt) * 128
                        sg = sps.tile([128, 1024], F32, tag="s", name=f"s{bb}_{jt}")
                        for h in range(2):
                            nc.tensor.matmul(
                                sg[:, h * 512 : (h + 1) * 512],
                                kTt[h * 64 : (h + 1) * 64, j0 : j0 + 128],
                                qT[h * 64 : (h + 1) * 64, i0 : i0 + 512],
                                start=True,
                                stop=True,
                                tile_position=(h * 64, 0),
                            )
                        et = ep.tile([128, 1024], BF16, tag="e", name=f"e{bb}_{jt}")
                        if jt in schraud_jt:
                            nc.vector.tensor_scalar(
                                et[:].bitcast(I16),
                                sg[:],
                                SCHRAUD_A,
                                SCHRAUD_B,
                                mult,
                                add,
                            )
                        else:
                            nc.scalar.activation(et[:], sg[:], Exp, scale=SCALE)
                        for h, vv in enumerate((v1, v2)):
                            nc.tensor.matmul(
                                pvs[h][:],
                                vv[:, jv : jv + 128],
                                et[:, h * 512 : (h + 1) * 512],
                                start=(jt == 0),
                                stop=(jt == 15),
                            )
                        if prev is not None and jt in (3, 5, 7, 9):
                            emit_po(prev, (jt - 3) // 2)
                    # --- evacuate block: normalized attn^T via den rows ---
                    at = atp.tile([128, 512], F16, tag="at", name=f"at{bb}")
                    for h in range(2):
                        # rec = 1/den via int-magic seed + one Newton step
                        y0 = rcp.tile([64, 512], F32, tag="rc", name=f"y0{bb}_{h}")
                        nc.vector.tensor_scalar(
                            y0[:].bitcast(I32),
                            pvs[h][64:128, :].bitcast(I32),
                            -1,
                            RECIP_K,
                            mult,
                            add,
                        )
                        u = rcp.tile([64, 512], F32, tag="rc", name=f"u{bb}_{h}")
                        nc.vector.scalar_tensor_tensor(
                            u[:], pvs[h][64:128, :], -1.0, y0[:], mult, mult
                        )
                        rec = rcp.tile([64, 512], F32, tag="rc", name=f"rc{bb}_{h}")
                        nc.vector.scalar_tensor_tensor(
                            rec[:], u[:], 2.0, y0[:], add, mult
                        )
                        nc.vector.tensor_tensor(
                            at[h * 64 : (h + 1) * 64, :],
                            pvs[h][0:64, :],
                            rec[:],
                            mult,
                        )
                    prev = (bb, at)

                # trailing po: block 7's scores pool is idle, use wide
                # 2-bank tiles there to avoid the 1-bank ring serialization
                pb, at = prev
                for tg in range(4):
                    row = pb * 512 + tg * 128
                    po = sps.tile([128, 1024], F32, tag="s", name=f"pot{tg}")
                    for oh in range(2):
                        nc.tensor.matmul(
                            po[:, oh * 512 : (oh + 1) * 512],
                            at[:, tg * 128 : (tg + 1) * 128],
                            wo_g[:, oh * 512 : (oh + 1) * 512],
                            start=True,
                            stop=True,
                        )
                    ob = obp.tile([128, 1024], F16, tag="ob", name=f"obt{tg}")
                    if tg % 2 == 0:
                        nc.scalar.activation(ob[:], po[:], Copy)
                    else:
                        nc.vector.tensor_copy(ob[:], po[:])
                    nc.sync.dma_start(po_d[row : row + 128, :], ob[:])

    nc.compile()
    return nc


_NC = None


def _get_nc():
    global _NC
    if _NC is None:
        _NC = build_nc()
    return _NC


def _gate(mask):
    """Exact jax fp32 gate: sigmoid(m) > 0.5 (fp32 logistic rounding)."""
    mask = np.asarray(mask, dtype=np.float32)
    return (np.float32(1.0) / (np.float32(1.0) + np.exp(-mask))) > np.float32(0.5)


def make_in_maps(x, qkv_weight, qkv_weight_mask, out_weight, out_weight_mask):
    x = np.asarray(x, dtype=np.float32)
    wqkv = np.where(_gate(qkv_weight_mask), np.asarray(qkv_weight, np.float32), 0.0)
    wo = np.where(_gate(out_weight_mask), np.asarray(out_weight, np.float32), 0.0)

    xT = np.ascontiguousarray(x.reshape(T, DIM).T.astype(np.float16))
    in_maps = []
    for c in range(NCORES):
        r0 = c * DV
        sl = slice(r0, r0 + DV)
        w_shard = np.concatenate(
            [wqkv[sl], wqkv[DIM + r0 : DIM + r0 + DV], wqkv[2 * DIM + r0 : 2 * DIM + r0 + DV]],
            axis=0,
        )  # [384, 1024] rows = (q h0,h1 | k h0,h1 | v h0,h1)
        in_maps.append(
            {
                "xT": xT,
                "wqkvT": np.ascontiguousarray(w_shard.T.astype(np.float16)),
                "woT": np.ascontiguousarray(wo[:, sl].T.astype(np.float16)),
            }
        )
    return in_maps


LAST_RESULTS = None  # BassKernelResults of the most recent run (for profiling)


def kernel(
    x,
    qkv_weight,
    qkv_weight_mask,
    out_weight,
    out_weight_mask,
    out_bias,
    out_bias_mask,
    _trace=False,
    _tmpdir=None,
):
    global LAST_RESULTS
    from concourse.bass_utils import run_bass_kernel_spmd

    nc = _get_nc()
    in_maps = make_in_maps(x, qkv_weight, qkv_weight_mask, out_weight, out_weight_mask)
    res = run_bass_kernel_spmd(
        nc, in_maps, list(range(NCORES)), trace=_trace, tmpdir=_tmpdir
    )
    LAST_RESULTS = res
    out = np.zeros((T, DIM), dtype=np.float32)
    for r in res.results:
        out += np.asarray(r["po"]).astype(np.float32)
    out_bias = np.asarray(out_bias, dtype=np.float32)
    out += np.where(_gate(out_bias_mask), out_bias, 0.0)[None, :]
    return out.reshape(B, N, DIM)
